# revision 31
# baseline (speedup 1.0000x reference)
"""Trainium2 Bass kernel for nn_MultiHeadLocalAttention (B=2,S=2048,W=32,D=1024,H=16).

Sharding: query-chunk parallel over 8 cores. Core c handles query rows
s' in [c*256, (c+1)*256) for both batches. Because of the reference's raw
.view on the k/v projections, head h of query s' reads k rows
s = h*128 + s'//16, w = 2*(s' mod 16) + w'//16 — i.e. core c needs exactly
k/v rows {h*128 + c*16 + j : h in [0,16), j in [0,16)}, giving a perfect
8-way split of the dominant k/v projection GEMMs (34 GFLOP each per core).

Per-core device row order for k/v is (b, h, p, j, i) where the original
(s, w) maps as s = h*128 + c*16 + j, w = 2i + p. With that order:
  - scores row (p,j,i) needs q_proj row u = 128*jh + 16j' + i (j = 8jh+j'),
    which is the SAME partition index in the matching q tile — no gather.
  - softmax over h is a free-dim reduce over per-(b,h) score planes.
  - the p-pair sum of the attention-weighted v partials is a PSUM
    accumulation of parity-selector matmuls that simultaneously produce
    the transposed layout needed by the output projection.

The k projection runs in fp8-e4m3 with DoubleRow (2 fp8 MACs/PE cell,
measured ~1.9x over 16-bit): score errors are damped by the head-softmax
so e4m3 keeps rel err under the gate; the v path stays 16-bit (its
errors pass straight through to the output). k is scaled x16 and Wk x256
on the host to keep e4m3 quantization away from subnormals; the 1/4096
is undone by the PSUM->SBUF copy scale. All 16-bit tensors use fp16
(not bf16) for the 3 extra mantissa bits — same PE/DVE speed. The
1/sqrt(hd) scale is folded into Wq/bq on the host.
"""
import sys
import types

sys.path.insert(0, "/opt/trn_rl_repo")

import numpy as np
import ml_dtypes

import concourse.bass as bass
import concourse.mybir as mybir
import concourse.tile as tile
from concourse.bass_utils import run_bass_kernel_spmd

BF16 = mybir.dt.bfloat16
F16 = mybir.dt.float16
F32 = mybir.dt.float32
FP8 = mybir.dt.float8e4
NPBF = ml_dtypes.bfloat16
NPF16 = np.float16
NPF8 = ml_dtypes.float8_e4m3
DR = mybir.MatmulPerfMode.DoubleRow

B, S, W, D, H = 2, 2048, 32, 1024, 16
P = 128
NCORES = 8
ROWS = B * H * 2 * 16 * 16  # 16384 k/v rows per core, order (b,h,p,j,i)
NBH = B * H  # 32 (b,h) blocks of 512 rows

K_SCALE = 16.0    # k quantization pre-scale for e4m3
WK_SCALE = 256.0  # Wk quantization pre-scale for e4m3


class _TC(tile.TileContext):
    """Walrus in this env rejects instructions carrying >1 sem wait (its
    setupSyncWait has a single wait slot). Two fixes at context exit:
    1. split any multi-wait instruction: excess waits move to same-engine
       NOPs inserted immediately before it (per-engine stream order makes
       this equivalent);
    2. emit the exit-drain's waits as individual SP wait_ge instructions
       instead of on the Drain itself."""

    def _split_multi_waits(self):
        nc = self.nc
        eng = {
            mybir.EngineType.PE: nc.tensor,
            mybir.EngineType.DVE: nc.vector,
            mybir.EngineType.Activation: nc.scalar,
            mybir.EngineType.Pool: nc.gpsimd,
            mybir.EngineType.SP: nc.sync,
        }
        end_bb = nc.cur_bb.bb
        for f in nc.m.functions:
            for blk in f.blocks:
                snapshot = list(blk.instructions)
                inserts = {}
                created = []
                for idx, ins in enumerate(snapshot):
                    si = getattr(ins, "sync_info", None)
                    if si is None or not si.on_wait or len(si.on_wait) <= 1:
                        continue
                    waits = list(si.on_wait)
                    nops = []
                    for w in waits[:-1]:
                        assert w.wait_reg is None, "register wait can't be split"
                        b = eng[ins.engine].nop()  # appends to end_bb
                        b.ins.sync_info = mybir.SyncInfo(on_wait=[w], on_update=[])
                        nops.append(b.ins)
                        created.append(b.ins)
                    si.on_wait = waits[-1:]
                    inserts[idx] = nops
                if not inserts:
                    continue
                created_ids = {id(n) for n in created}
                # pull the freshly-appended nops back out of the end block
                end_bb.instructions = [
                    i for i in end_bb.instructions if id(i) not in created_ids
                ]
                out = []
                for idx, ins in enumerate(snapshot):
                    out.extend(inserts.get(idx, ()))
                    out.append(ins)
                blk.instructions = out

    def _drain_and_barrier(self, tick_clock, wait_clock):
        self._split_multi_waits()
        gc = tick_clock.global_clock
        for proc, sem in sorted(wait_clock.sems.allocated().items()):
            ticks = gc.peek_next(proc) - 1
            if ticks > 0:
                val = ticks * (16 if sem.name.startswith("DMA") else 1)
                self.nc.sync.wait_ge(sem, val)
        self.nc.sync.drain()
        self.nc.all_engine_barrier()
        popped = self.nc._tile_sem_poison_stack.pop()
        assert popped is self._sem_poison
        self.nc.clear_and_free_semaphores(list(self.sems.allocated().values()))
        self.nc.all_engine_barrier()


def _build_nc(with_bias=True):
    import contextlib

    nc = bass.Bass()
    kTb = nc.dram_tensor("kTb", [P, NBH, 8, 512], FP8, kind="ExternalInput")
    vTb = nc.dram_tensor("vTb", [P, NBH, 8, 512], F16, kind="ExternalInput")
    qTb = nc.dram_tensor("qTb", [P, 8, 512], F16, kind="ExternalInput")
    wqd = nc.dram_tensor("wq", [P, 8, D], F16, kind="ExternalInput")
    wkd = nc.dram_tensor("wk", [P, 8, D], FP8, kind="ExternalInput")
    wvd = nc.dram_tensor("wv", [P, 8, D], F16, kind="ExternalInput")
    wod = nc.dram_tensor("wo", [P, 8, D], F16, kind="ExternalInput")
    bqd = nc.dram_tensor("bq", [D], F32, kind="ExternalInput")
    bkd = nc.dram_tensor("bk", [D], F32, kind="ExternalInput")
    bvd = nc.dram_tensor("bv", [D], F32, kind="ExternalInput")
    bod = nc.dram_tensor("bo", [D], F32, kind="ExternalInput")
    sel2d = nc.dram_tensor("sel2", [P, 2, 64], F16, kind="ExternalInput")
    outd = nc.dram_tensor("out", [512, D], F16, kind="ExternalOutput")

    with _TC(nc) as tc, contextlib.ExitStack() as ex:
        wpool = ex.enter_context(tc.tile_pool(name="wts", bufs=1))
        persist = ex.enter_context(tc.tile_pool(name="persist", bufs=1))
        kblk = ex.enter_context(tc.tile_pool(name="kblk", bufs=4))
        vblk = ex.enter_context(tc.tile_pool(name="vblk", bufs=3))
        actp = ex.enter_context(tc.tile_pool(name="act", bufs=16))
        prodp = ex.enter_context(tc.tile_pool(name="prod", bufs=5))
        miscp = ex.enter_context(tc.tile_pool(name="misc", bufs=4))
        smallp = ex.enter_context(tc.tile_pool(name="small", bufs=12))
        gps = ex.enter_context(tc.tile_pool(name="gps", bufs=6 if not with_bias else 4,
                                            space="PSUM"))
        pps = ex.enter_context(tc.tile_pool(name="pps", bufs=2, space="PSUM"))

        # ---- constants -------------------------------------------------
        # chunked weight loads: one dma per f-chunk so transfers spread
        # across the 16 DMA queues and the first matmul isn't gated on a
        # single 2MB transfer draining through one queue
        def load_w(dram_t, name, dt=F16):
            t = wpool.tile([P, 8, D], dt, name=name)
            for fc in range(8):
                nc.sync.dma_start(t[:, fc], dram_t[:, fc])
            return t

        def load_b(dram_t, name):
            t = wpool.tile([P, D], F32, name=name)
            nc.sync.dma_start(t[:], dram_t[:].partition_broadcast(P))
            return t

        # DMA waits are cumulative in issue order, so emit only what the
        # q projection needs first; wk/sel before the K loop; wv/wo and
        # the v prefetch are interleaved into the K loop emission below.
        # partition-sliced loads: per-partition rows are contiguous in DRAM,
        # so a 16-partition slice is 16 fat descriptors instead of 128 thin
        # ones — the startup path is DMA-descriptor-bound, not byte-bound
        qts = persist.tile([P, 8, 512], F16, name="qts")
        for p0 in range(0, P, 16):
            nc.sync.dma_start(qts[p0:p0 + 16], qTb[p0:p0 + 16])
        wq = wpool.tile([P, 8, D], F16, name="wq")
        for p0 in range(0, P, 16):
            nc.sync.dma_start(wq[p0:p0 + 16], wqd[p0:p0 + 16])
        if with_bias:
            bq = load_b(bqd, "bq")

        # ---- q projection ---------------------------------------------
        qp = persist.tile([P, 4, D], F16, name="qp")  # u-chunks (b, jh)
        for t in range(4):
            for oh in range(2):
                ps = gps.tile([P, 512], F32, name="gps")
                for fc in range(8):
                    nc.tensor.matmul(
                        ps[:],
                        qts[:, fc, 128 * t:128 * t + 128],
                        wq[:, fc, 512 * oh:512 * oh + 512],
                        start=(fc == 0),
                        stop=(fc == 7),
                    )
                if with_bias:
                    nc.vector.tensor_add(
                        qp[:, t, 512 * oh:512 * oh + 512],
                        ps[:],
                        bq[:, 512 * oh:512 * oh + 512],
                    )
                else:
                    nc.scalar.copy(qp[:, t, 512 * oh:512 * oh + 512], ps[:])

        # ---- score planes / attn planes (persistent) -------------------
        planes = [[persist.tile([P, H, 16], F32, name=f"pl_{b}_{t}")
                   for t in range(4)] for b in range(B)]
        attns = [[persist.tile([P, H, 16], F16, name=f"at_{b}_{t}")
                  for t in range(4)] for b in range(B)]
        # transposed out_local, chunked by contraction block g and row-chunk:
        # olT[rc][cl, g, r] = out_local[row 128*rc + r, c = 128*g + cl]
        olTs = [persist.tile([P, 8, P], F16, name=f"olT{rc}")
                for rc in range(4)]

        kpre = {}
        for kb in range(3):
            tk = kblk.tile([P, 8, 512], FP8, name="kblk")
            nc.sync.dma_start(tk[:], kTb[:, kb])
            kpre[kb] = tk
        wk = load_w(wkd, "wk", FP8)
        sel2 = wpool.tile([P, 2, 64], F16, name="sel2")
        nc.sync.dma_start(sel2[:], sel2d[:])
        if with_bias:
            bk = load_b(bkd, "bk")
        vpre = {}

        # ---- K phase: fp8 DoubleRow projections + scores ---------------
        for b in range(B):
            for h in range(H):
                bh = b * H + h
                if bh in kpre:
                    ts_ = kpre.pop(bh)
                else:
                    ts_ = kblk.tile([P, 8, 512], FP8, name="kblk")
                    nc.sync.dma_start(ts_[:], kTb[:, bh])
                if bh == 2:
                    # prefetch the first v blocks + V-phase weights now:
                    # late enough not to gate the K start, early enough
                    # to be resident long before the V phase begins
                    for vb in range(2):
                        tv = vblk.tile([P, 8, 512], F16, name="vblk")
                        nc.sync.dma_start(tv[:], vTb[:, vb])
                        vpre[vb] = tv
                    wv = load_w(wvd, "wv")
                    wo = load_w(wod, "wo")
                    if with_bias:
                        bv = load_b(bvd, "bv")
                        bo = load_b(bod, "bo")
                for t in range(4):
                    pss = [gps.tile([P, 512], F32, name="gps")
                           for _ in range(2)]
                    for fcp in range(4):
                        for oh in range(2):
                            nc.tensor.matmul(
                                pss[oh],
                                ts_[:, 2 * fcp:2 * fcp + 2,
                                    128 * t:128 * t + 128],
                                wk[:, 2 * fcp:2 * fcp + 2,
                                   512 * oh:512 * oh + 512],
                                start=(fcp == 0),
                                stop=(fcp == 3),
                                perf_mode=DR,
                            )
                    kv = actp.tile([P, D], F16, name="kv")
                    for oh in range(2):
                        if with_bias:
                            nc.vector.scalar_tensor_tensor(
                                kv[:, 512 * oh:512 * oh + 512],
                                pss[oh][:],
                                1.0 / (K_SCALE * WK_SCALE),
                                bk[:, 512 * oh:512 * oh + 512],
                                op0=mybir.AluOpType.mult,
                                op1=mybir.AluOpType.add,
                            )
                        else:
                            nc.scalar.mul(
                                kv[:, 512 * oh:512 * oh + 512], pss[oh][:],
                                1.0 / (K_SCALE * WK_SCALE))
                    jh = t % 2
                    qsm = (qp[:, 2 * b + jh, 64 * h:64 * h + 64][:, None, :])
                    prod = prodp.tile([P, 16, 64], F16, name="prod")
                    nc.vector.tensor_mul(
                        prod[:],
                        kv[:].rearrange("p (c e) -> p c e", c=16),
                        qsm.broadcast_to([P, 16, 64]),
                    )
                    nc.vector.tensor_reduce(
                        planes[b][t][:, h, :],
                        prod[:],
                        axis=mybir.AxisListType.X,
                        op=mybir.AluOpType.add,
                    )

        # ---- softmax over heads ----------------------------------------
        for b in range(B):
            for t in range(4):
                pl = planes[b][t]
                mx = smallp.tile([P, 16], F32, name="mx")
                nc.vector.tensor_reduce(
                    mx[:], pl[:].rearrange("p h c -> p c h"),
                    axis=mybir.AxisListType.X, op=mybir.AluOpType.max,
                )
                sub = miscp.tile([P, H, 16], F32, name="sm")
                nc.vector.tensor_sub(
                    sub[:], pl[:],
                    mx[:][:, None, :].broadcast_to([P, H, 16]),
                )
                epl = miscp.tile([P, H, 16], F32, name="ep")
                nc.scalar.activation(
                    epl[:], sub[:], mybir.ActivationFunctionType.Exp,
                )
                z = smallp.tile([P, 16], F32, name="z")
                nc.vector.tensor_reduce(
                    z[:], epl[:].rearrange("p h c -> p c h"),
                    axis=mybir.AxisListType.X, op=mybir.AluOpType.add,
                )
                rz = smallp.tile([P, 16], F32, name="rz")
                nc.vector.reciprocal(rz[:], z[:])
                nc.vector.tensor_mul(
                    attns[b][t][:], epl[:],
                    rz[:][:, None, :].broadcast_to([P, H, 16]),
                )

        # ---- V phase: fp16 projections + weighted sum + repack ---------
        def out_proj(rc):
            for oh in range(2):
                ps = gps.tile([P, 512], F32, name="gps")
                for g in range(8):
                    nc.tensor.matmul(
                        ps[:],
                        olTs[rc][:, g, :],
                        wo[:, g, 512 * oh:512 * oh + 512],
                        start=(g == 0),
                        stop=(g == 7),
                    )
                for half in range(2):
                    osb = miscp.tile([P, 256], F16, name="osb")
                    sl = slice(256 * half, 256 * half + 256)
                    if with_bias:
                        nc.vector.tensor_add(
                            osb[:], ps[:, sl],
                            bo[:, 512 * oh + 256 * half:
                                512 * oh + 256 * half + 256])
                    else:
                        nc.scalar.copy(osb[:], ps[:, sl])
                    nc.sync.dma_start(
                        outd[128 * rc:128 * rc + 128,
                             512 * oh + 256 * half:
                             512 * oh + 256 * half + 256],
                        osb[:],
                    )

        def repack(bh, parts):
            # emitted one block late: the PE streams the next block's GEMM
            # while the DVE finishes this block's parts
            for jh in range(2):
                aps = pps.tile([P, 64], F32, name="pps")
                for ti, t in enumerate((jh, 2 + jh)):
                    for q3 in range(2):
                        nc.tensor.matmul(
                            aps[:][64 * q3:64 * q3 + 64, :],
                            parts[t][:],
                            sel2[:, q3, :],
                            start=(ti == 0),
                            stop=(ti == 1),
                            tile_position=(0, 64 * q3),
                            skip_group_check=True,
                        )
                rb = bh * 16 + jh * 8
                nc.scalar.copy(
                    olTs[rb // P][:, :, rb % P:rb % P + 8],
                    aps[:].rearrange("p (g r) -> p g r", g=8),
                )

        pending = None
        for b in range(B):
            for h in range(H):
                bh = b * H + h
                if bh in vpre:
                    ts_ = vpre.pop(bh)
                else:
                    ts_ = vblk.tile([P, 8, 512], F16, name="vblk")
                    nc.sync.dma_start(ts_[:], vTb[:, bh])
                # delayed output projection: emit one block late so the PE
                # streams this block's GEMM while Act finishes olT copies
                if bh % 8 == 1 and bh > 8:
                    out_proj(bh // 8 - 1)
                parts = []
                for t in range(4):
                    pss = [gps.tile([P, 512], F32, name="gps")
                           for _ in range(2)]
                    for fc in range(8):
                        for oh in range(2):
                            nc.tensor.matmul(
                                pss[oh],
                                ts_[:, fc, 128 * t:128 * t + 128],
                                wv[:, fc, 512 * oh:512 * oh + 512],
                                start=(fc == 0),
                                stop=(fc == 7),
                            )
                    prod2 = prodp.tile([P, 64, 16], F16, name="pr2")
                    vv = actp.tile([P, D], F16, name="kv")
                    for oh in range(2):
                        if with_bias:
                            nc.vector.tensor_add(
                                vv[:, 512 * oh:512 * oh + 512],
                                pss[oh][:],
                                bv[:, 512 * oh:512 * oh + 512],
                            )
                        else:
                            nc.scalar.copy(
                                vv[:, 512 * oh:512 * oh + 512], pss[oh][:])
                    for c0 in (0, 8):
                        nc.vector.tensor_mul(
                            prod2[:, :, c0:c0 + 8],
                            vv[:, 512 * (c0 // 8):512 * (c0 // 8) + 512]
                            .rearrange("p (c e) -> p e c", c=8),
                            attns[b][t][:, h, c0:c0 + 8][:, None, :]
                            .broadcast_to([P, 64, 8]),
                        )
                    part = smallp.tile([P, 64], F16, name="part")
                    with nc.allow_low_precision("single output rounding"):
                        nc.vector.tensor_reduce(
                            part[:], prod2[:],
                            axis=mybir.AxisListType.X, op=mybir.AluOpType.add,
                        )
                    parts.append(part)
                if pending is not None:
                    repack(*pending)
                pending = (bh, parts)
        repack(*pending)
        out_proj(3)

    return nc


_NC_CACHE = {}


def _get_nc(with_bias=True):
    if with_bias not in _NC_CACHE:
        _NC_CACHE[with_bias] = _build_nc(with_bias)
    return _NC_CACHE[with_bias]


def _host_prep(q, k, v, Wq, bq, Wk, bk, Wv, bv, Wo, bo):
    """Build the 8 per-core input maps (k fp8, v/q/weights bf16, biases f32)."""
    q = np.asarray(q, np.float32)
    k = np.asarray(k, np.float32)
    v = np.asarray(v, np.float32)
    # fold 1/sqrt(hd) into the q path (fp8 scales are undone on-device)
    Wqs = np.asarray(Wq, np.float32) * 0.125
    bqs = np.asarray(bq, np.float32) * 0.125

    def wprep(Wm, scale=1.0, dt=NPF16, clip=None):
        # [P, 8, D]: [f_local, f_chunk, o] of W.T
        WT = np.ascontiguousarray(np.asarray(Wm, np.float32).T) * scale  # [f, o]
        if clip is not None:
            WT = np.clip(WT, -clip, clip)
        return np.ascontiguousarray(
            WT.reshape(8, P, D).transpose(1, 0, 2)).astype(dt)

    wq_b = wprep(Wqs)
    wk_b = wprep(Wk, WK_SCALE, NPF8, 240.0)
    wv_b = wprep(Wv)
    wo_b = wprep(Wo)

    i_idx = np.arange(P)  # rows (j', i) packing for sel2
    r_i = i_idx % 16
    r_j = i_idx // 16
    g_col = np.arange(64) // 8
    j_col = np.arange(64) % 8
    # sel2[p=(j',i), q, (g,r)] = (j'==r) & (i == 2g+q)
    sel2 = np.stack([
        ((r_j[:, None] == j_col[None, :]) &
         (r_i[:, None] == 2 * g_col[None, :] + qq)).astype(NPF16)
        for qq in range(2)
    ], axis=1)  # [P, 2, 64]

    jj = np.arange(16)
    hh = np.arange(H)
    in_maps = []
    for c in range(NCORES):
        srows = hh[:, None] * 128 + c * 16 + jj[None, :]  # [h, j]

        def kvprep(x, dt, scale):
            xs = x[:, srows]  # [b, h, j, w, d]
            xs = xs.reshape(B, H, 16, 16, 2, D).transpose(0, 1, 4, 2, 3, 5)
            flat = xs.reshape(ROWS, D)  # rows (b,h,p,j,i)
            # kTb[f_local, bh_block, f_chunk, r'] where block rows r' in [0,512)
            xT = flat.T * scale  # [D, ROWS]
            if dt is NPF8:
                xT = np.clip(xT, -240.0, 240.0)
            xT = xT.astype(dt).reshape(8, P, NBH, 512)  # fc, f, blk, r
            return np.ascontiguousarray(xT.transpose(1, 2, 0, 3))  # f, blk, fc, r

        qc = q[:, c * 256:(c + 1) * 256, :].reshape(512, D)
        qT = qc.T.astype(NPF16).reshape(8, P, 512)
        qTb = np.ascontiguousarray(qT.transpose(1, 0, 2))  # [P, 8, 512]

        in_maps.append({
            "kTb": kvprep(k, NPF8, K_SCALE),
            "vTb": kvprep(v, NPF16, 1.0),
            "qTb": qTb,
            "wq": wq_b, "wk": wk_b, "wv": wv_b, "wo": wo_b,
            "bq": bqs,
            "bk": np.asarray(bk, np.float32),
            "bv": np.asarray(bv, np.float32), "bo": np.asarray(bo, np.float32),
            "sel2": sel2,
        })
    return in_maps


def kernel(q, k, v, Wq, bq, Wk, bk, Wv, bv, Wo, bo, num_heads, _trace=False):
    assert int(num_heads) == H
    with_bias = any(
        np.any(np.asarray(x, np.float32)) for x in (bq, bk, bv, bo))
    nc = _get_nc(with_bias)
    in_maps = _host_prep(q, k, v, Wq, bq, Wk, bk, Wv, bv, Wo, bo)
    res = run_bass_kernel_spmd(nc, in_maps, core_ids=list(range(NCORES)),
                               trace=_trace)
    full = np.zeros((B, S, D), np.float32)
    for c in range(NCORES):
        oc = res.results[c]["out"].reshape(B, H, 16, D)
        for h in range(H):
            full[:, h * 128 + c * 16: h * 128 + c * 16 + 16, :] = oc[:, h]
    if _trace:
        kernel._last_exec_ns = res.exec_time_ns
        kernel._last_results = res
    return full


# revision 32
# speedup vs baseline: 1.0018x; 1.0018x over previous
"""Trainium2 Bass kernel for nn_MultiHeadLocalAttention (B=2,S=2048,W=32,D=1024,H=16).

Sharding: query-chunk parallel over 8 cores. Core c handles query rows
s' in [c*256, (c+1)*256) for both batches. Because of the reference's raw
.view on the k/v projections, head h of query s' reads k rows
s = h*128 + s'//16, w = 2*(s' mod 16) + w'//16 — i.e. core c needs exactly
k/v rows {h*128 + c*16 + j : h in [0,16), j in [0,16)}, giving a perfect
8-way split of the dominant k/v projection GEMMs (34 GFLOP each per core).

Per-core device row order for k/v is (b, h, p, j, i) where the original
(s, w) maps as s = h*128 + c*16 + j, w = 2i + p. With that order:
  - scores row (p,j,i) needs q_proj row u = 128*jh + 16j' + i (j = 8jh+j'),
    which is the SAME partition index in the matching q tile — no gather.
  - softmax over h is a free-dim reduce over per-(b,h) score planes.
  - the p-pair sum of the attention-weighted v partials is a PSUM
    accumulation of parity-selector matmuls that simultaneously produce
    the transposed layout needed by the output projection.

The k projection runs in fp8-e4m3 with DoubleRow (2 fp8 MACs/PE cell,
measured ~1.9x over 16-bit): score errors are damped by the head-softmax
so e4m3 keeps rel err under the gate; the v path stays 16-bit (its
errors pass straight through to the output). k is scaled x16 and Wk x256
on the host to keep e4m3 quantization away from subnormals; the 1/4096
is undone by the PSUM->SBUF copy scale. All 16-bit tensors use fp16
(not bf16) for the 3 extra mantissa bits — same PE/DVE speed. The
1/sqrt(hd) scale is folded into Wq/bq on the host.
"""
import sys
import types

sys.path.insert(0, "/opt/trn_rl_repo")

import numpy as np
import ml_dtypes

import concourse.bass as bass
import concourse.mybir as mybir
import concourse.tile as tile
from concourse.bass_utils import run_bass_kernel_spmd

BF16 = mybir.dt.bfloat16
F16 = mybir.dt.float16
F32 = mybir.dt.float32
FP8 = mybir.dt.float8e4
NPBF = ml_dtypes.bfloat16
NPF16 = np.float16
NPF8 = ml_dtypes.float8_e4m3
DR = mybir.MatmulPerfMode.DoubleRow

B, S, W, D, H = 2, 2048, 32, 1024, 16
P = 128
NCORES = 8
ROWS = B * H * 2 * 16 * 16  # 16384 k/v rows per core, order (b,h,p,j,i)
NBH = B * H  # 32 (b,h) blocks of 512 rows

K_SCALE = 16.0    # k quantization pre-scale for e4m3
WK_SCALE = 256.0  # Wk quantization pre-scale for e4m3


class _TC(tile.TileContext):
    """Walrus in this env rejects instructions carrying >1 sem wait (its
    setupSyncWait has a single wait slot). Two fixes at context exit:
    1. split any multi-wait instruction: excess waits move to same-engine
       NOPs inserted immediately before it (per-engine stream order makes
       this equivalent);
    2. emit the exit-drain's waits as individual SP wait_ge instructions
       instead of on the Drain itself."""

    def _split_multi_waits(self):
        nc = self.nc
        eng = {
            mybir.EngineType.PE: nc.tensor,
            mybir.EngineType.DVE: nc.vector,
            mybir.EngineType.Activation: nc.scalar,
            mybir.EngineType.Pool: nc.gpsimd,
            mybir.EngineType.SP: nc.sync,
        }
        end_bb = nc.cur_bb.bb
        for f in nc.m.functions:
            for blk in f.blocks:
                snapshot = list(blk.instructions)
                inserts = {}
                created = []
                for idx, ins in enumerate(snapshot):
                    si = getattr(ins, "sync_info", None)
                    if si is None or not si.on_wait or len(si.on_wait) <= 1:
                        continue
                    waits = list(si.on_wait)
                    nops = []
                    for w in waits[:-1]:
                        assert w.wait_reg is None, "register wait can't be split"
                        b = eng[ins.engine].nop()  # appends to end_bb
                        b.ins.sync_info = mybir.SyncInfo(on_wait=[w], on_update=[])
                        nops.append(b.ins)
                        created.append(b.ins)
                    si.on_wait = waits[-1:]
                    inserts[idx] = nops
                if not inserts:
                    continue
                created_ids = {id(n) for n in created}
                # pull the freshly-appended nops back out of the end block
                end_bb.instructions = [
                    i for i in end_bb.instructions if id(i) not in created_ids
                ]
                out = []
                for idx, ins in enumerate(snapshot):
                    out.extend(inserts.get(idx, ()))
                    out.append(ins)
                blk.instructions = out

    def _drain_and_barrier(self, tick_clock, wait_clock):
        self._split_multi_waits()
        gc = tick_clock.global_clock
        for proc, sem in sorted(wait_clock.sems.allocated().items()):
            ticks = gc.peek_next(proc) - 1
            if ticks > 0:
                val = ticks * (16 if sem.name.startswith("DMA") else 1)
                self.nc.sync.wait_ge(sem, val)
        self.nc.sync.drain()
        self.nc.all_engine_barrier()
        popped = self.nc._tile_sem_poison_stack.pop()
        assert popped is self._sem_poison
        self.nc.clear_and_free_semaphores(list(self.sems.allocated().values()))
        self.nc.all_engine_barrier()


def _build_nc(with_bias=True):
    import contextlib

    nc = bass.Bass()
    kTb = nc.dram_tensor("kTb", [P, NBH, 8, 512], FP8, kind="ExternalInput")
    vTb = nc.dram_tensor("vTb", [P, NBH, 8, 512], F16, kind="ExternalInput")
    qTb = nc.dram_tensor("qTb", [P, 8, 512], F16, kind="ExternalInput")
    wqd = nc.dram_tensor("wq", [P, 8, D], F16, kind="ExternalInput")
    wkd = nc.dram_tensor("wk", [P, 8, D], FP8, kind="ExternalInput")
    wvd = nc.dram_tensor("wv", [P, 8, D], F16, kind="ExternalInput")
    wod = nc.dram_tensor("wo", [P, 8, D], F16, kind="ExternalInput")
    bqd = nc.dram_tensor("bq", [D], F32, kind="ExternalInput")
    bkd = nc.dram_tensor("bk", [D], F32, kind="ExternalInput")
    bvd = nc.dram_tensor("bv", [D], F32, kind="ExternalInput")
    bod = nc.dram_tensor("bo", [D], F32, kind="ExternalInput")
    sel2d = nc.dram_tensor("sel2", [P, 2, 64], F16, kind="ExternalInput")
    outd = nc.dram_tensor("out", [512, D], F16, kind="ExternalOutput")

    with _TC(nc) as tc, contextlib.ExitStack() as ex:
        wpool = ex.enter_context(tc.tile_pool(name="wts", bufs=1))
        persist = ex.enter_context(tc.tile_pool(name="persist", bufs=1))
        kblk = ex.enter_context(tc.tile_pool(name="kblk", bufs=5))
        vblk = ex.enter_context(tc.tile_pool(name="vblk", bufs=3))
        actp = ex.enter_context(tc.tile_pool(name="act", bufs=16))
        prodp = ex.enter_context(tc.tile_pool(name="prod", bufs=5))
        miscp = ex.enter_context(tc.tile_pool(name="misc", bufs=4))
        smallp = ex.enter_context(tc.tile_pool(name="small", bufs=12))
        gps = ex.enter_context(tc.tile_pool(name="gps", bufs=6 if not with_bias else 4,
                                            space="PSUM"))
        pps = ex.enter_context(tc.tile_pool(name="pps", bufs=2, space="PSUM"))

        # ---- constants -------------------------------------------------
        # chunked weight loads: one dma per f-chunk so transfers spread
        # across the 16 DMA queues and the first matmul isn't gated on a
        # single 2MB transfer draining through one queue
        def load_w(dram_t, name, dt=F16):
            t = wpool.tile([P, 8, D], dt, name=name)
            for fc in range(8):
                nc.sync.dma_start(t[:, fc], dram_t[:, fc])
            return t

        def load_b(dram_t, name):
            t = wpool.tile([P, D], F32, name=name)
            nc.sync.dma_start(t[:], dram_t[:].partition_broadcast(P))
            return t

        # DMA waits are cumulative in issue order, so emit only what the
        # q projection needs first; wk/sel before the K loop; wv/wo and
        # the v prefetch are interleaved into the K loop emission below.
        # partition-sliced loads: per-partition rows are contiguous in DRAM,
        # so a 16-partition slice is 16 fat descriptors instead of 128 thin
        # ones — the startup path is DMA-descriptor-bound, not byte-bound
        qts = persist.tile([P, 8, 512], F16, name="qts")
        for p0 in range(0, P, 16):
            nc.sync.dma_start(qts[p0:p0 + 16], qTb[p0:p0 + 16])
        wq = wpool.tile([P, 8, D], F16, name="wq")
        for p0 in range(0, P, 16):
            nc.sync.dma_start(wq[p0:p0 + 16], wqd[p0:p0 + 16])
        if with_bias:
            bq = load_b(bqd, "bq")

        # ---- q projection ---------------------------------------------
        qp = persist.tile([P, 4, D], F16, name="qp")  # u-chunks (b, jh)
        for t in range(4):
            for oh in range(2):
                ps = gps.tile([P, 512], F32, name="gps")
                for fc in range(8):
                    nc.tensor.matmul(
                        ps[:],
                        qts[:, fc, 128 * t:128 * t + 128],
                        wq[:, fc, 512 * oh:512 * oh + 512],
                        start=(fc == 0),
                        stop=(fc == 7),
                    )
                if with_bias:
                    nc.vector.tensor_add(
                        qp[:, t, 512 * oh:512 * oh + 512],
                        ps[:],
                        bq[:, 512 * oh:512 * oh + 512],
                    )
                else:
                    nc.scalar.copy(qp[:, t, 512 * oh:512 * oh + 512], ps[:])

        # ---- score planes / attn planes (persistent) -------------------
        planes = [[persist.tile([P, H, 16], F32, name=f"pl_{b}_{t}")
                   for t in range(4)] for b in range(B)]
        attns = [[persist.tile([P, H, 16], F16, name=f"at_{b}_{t}")
                  for t in range(4)] for b in range(B)]
        # transposed out_local, chunked by contraction block g and row-chunk:
        # olT[rc][cl, g, r] = out_local[row 128*rc + r, c = 128*g + cl]
        olTs = [persist.tile([P, 8, P], F16, name=f"olT{rc}")
                for rc in range(4)]

        kpre = {}
        for kb in range(3):
            tk = kblk.tile([P, 8, 512], FP8, name="kblk")
            nc.sync.dma_start(tk[:], kTb[:, kb])
            kpre[kb] = tk
        wk = load_w(wkd, "wk", FP8)
        sel2 = wpool.tile([P, 2, 64], F16, name="sel2")
        nc.sync.dma_start(sel2[:], sel2d[:])
        if with_bias:
            bk = load_b(bkd, "bk")
        vpre = {}

        # ---- K phase: fp8 DoubleRow projections + scores ---------------
        for b in range(B):
            for h in range(H):
                bh = b * H + h
                if bh in kpre:
                    ts_ = kpre.pop(bh)
                else:
                    ts_ = kblk.tile([P, 8, 512], FP8, name="kblk")
                    nc.sync.dma_start(ts_[:], kTb[:, bh])
                if bh == 2:
                    # prefetch the first v blocks + V-phase weights now:
                    # late enough not to gate the K start, early enough
                    # to be resident long before the V phase begins
                    for vb in range(2):
                        tv = vblk.tile([P, 8, 512], F16, name="vblk")
                        nc.sync.dma_start(tv[:], vTb[:, vb])
                        vpre[vb] = tv
                    wv = load_w(wvd, "wv")
                    wo = load_w(wod, "wo")
                    if with_bias:
                        bv = load_b(bvd, "bv")
                        bo = load_b(bod, "bo")
                for t in range(4):
                    pss = [gps.tile([P, 512], F32, name="gps")
                           for _ in range(2)]
                    for fcp in range(4):
                        for oh in range(2):
                            nc.tensor.matmul(
                                pss[oh],
                                ts_[:, 2 * fcp:2 * fcp + 2,
                                    128 * t:128 * t + 128],
                                wk[:, 2 * fcp:2 * fcp + 2,
                                   512 * oh:512 * oh + 512],
                                start=(fcp == 0),
                                stop=(fcp == 3),
                                perf_mode=DR,
                            )
                    kv = actp.tile([P, D], F16, name="kv")
                    for oh in range(2):
                        if with_bias:
                            nc.vector.scalar_tensor_tensor(
                                kv[:, 512 * oh:512 * oh + 512],
                                pss[oh][:],
                                1.0 / (K_SCALE * WK_SCALE),
                                bk[:, 512 * oh:512 * oh + 512],
                                op0=mybir.AluOpType.mult,
                                op1=mybir.AluOpType.add,
                            )
                        else:
                            nc.scalar.mul(
                                kv[:, 512 * oh:512 * oh + 512], pss[oh][:],
                                1.0 / (K_SCALE * WK_SCALE))
                    jh = t % 2
                    qsm = (qp[:, 2 * b + jh, 64 * h:64 * h + 64][:, None, :])
                    prod = prodp.tile([P, 16, 64], F16, name="prod")
                    nc.vector.tensor_mul(
                        prod[:],
                        kv[:].rearrange("p (c e) -> p c e", c=16),
                        qsm.broadcast_to([P, 16, 64]),
                    )
                    nc.vector.tensor_reduce(
                        planes[b][t][:, h, :],
                        prod[:],
                        axis=mybir.AxisListType.X,
                        op=mybir.AluOpType.add,
                    )

        # ---- softmax over heads ----------------------------------------
        for b in range(B):
            for t in range(4):
                pl = planes[b][t]
                mx = smallp.tile([P, 16], F32, name="mx")
                nc.vector.tensor_reduce(
                    mx[:], pl[:].rearrange("p h c -> p c h"),
                    axis=mybir.AxisListType.X, op=mybir.AluOpType.max,
                )
                sub = miscp.tile([P, H, 16], F32, name="sm")
                nc.vector.tensor_sub(
                    sub[:], pl[:],
                    mx[:][:, None, :].broadcast_to([P, H, 16]),
                )
                epl = miscp.tile([P, H, 16], F32, name="ep")
                nc.scalar.activation(
                    epl[:], sub[:], mybir.ActivationFunctionType.Exp,
                )
                z = smallp.tile([P, 16], F32, name="z")
                nc.vector.tensor_reduce(
                    z[:], epl[:].rearrange("p h c -> p c h"),
                    axis=mybir.AxisListType.X, op=mybir.AluOpType.add,
                )
                rz = smallp.tile([P, 16], F32, name="rz")
                nc.vector.reciprocal(rz[:], z[:])
                nc.vector.tensor_mul(
                    attns[b][t][:], epl[:],
                    rz[:][:, None, :].broadcast_to([P, H, 16]),
                )

        # ---- V phase: fp16 projections + weighted sum + repack ---------
        def out_proj(rc):
            for oh in range(2):
                ps = gps.tile([P, 512], F32, name="gps")
                for g in range(8):
                    nc.tensor.matmul(
                        ps[:],
                        olTs[rc][:, g, :],
                        wo[:, g, 512 * oh:512 * oh + 512],
                        start=(g == 0),
                        stop=(g == 7),
                    )
                for half in range(2):
                    osb = miscp.tile([P, 256], F16, name="osb")
                    sl = slice(256 * half, 256 * half + 256)
                    if with_bias:
                        nc.vector.tensor_add(
                            osb[:], ps[:, sl],
                            bo[:, 512 * oh + 256 * half:
                                512 * oh + 256 * half + 256])
                    else:
                        nc.scalar.copy(osb[:], ps[:, sl])
                    nc.sync.dma_start(
                        outd[128 * rc:128 * rc + 128,
                             512 * oh + 256 * half:
                             512 * oh + 256 * half + 256],
                        osb[:],
                    )

        def repack(bh, parts):
            # emitted one block late: the PE streams the next block's GEMM
            # while the DVE finishes this block's parts
            for jh in range(2):
                aps = pps.tile([P, 64], F32, name="pps")
                for ti, t in enumerate((jh, 2 + jh)):
                    for q3 in range(2):
                        nc.tensor.matmul(
                            aps[:][64 * q3:64 * q3 + 64, :],
                            parts[t][:],
                            sel2[:, q3, :],
                            start=(ti == 0),
                            stop=(ti == 1),
                            tile_position=(0, 64 * q3),
                            skip_group_check=True,
                        )
                rb = bh * 16 + jh * 8
                nc.scalar.copy(
                    olTs[rb // P][:, :, rb % P:rb % P + 8],
                    aps[:].rearrange("p (g r) -> p g r", g=8),
                )

        pending = None
        for b in range(B):
            for h in range(H):
                bh = b * H + h
                if bh in vpre:
                    ts_ = vpre.pop(bh)
                else:
                    ts_ = vblk.tile([P, 8, 512], F16, name="vblk")
                    nc.sync.dma_start(ts_[:], vTb[:, bh])
                # delayed output projection: emit one block late so the PE
                # streams this block's GEMM while Act finishes olT copies
                if bh % 8 == 1 and bh > 8:
                    out_proj(bh // 8 - 1)
                parts = []
                for t in range(4):
                    pss = [gps.tile([P, 512], F32, name="gps")
                           for _ in range(2)]
                    for fc in range(8):
                        for oh in range(2):
                            nc.tensor.matmul(
                                pss[oh],
                                ts_[:, fc, 128 * t:128 * t + 128],
                                wv[:, fc, 512 * oh:512 * oh + 512],
                                start=(fc == 0),
                                stop=(fc == 7),
                            )
                    prod2 = prodp.tile([P, 64, 16], F16, name="pr2")
                    vv = actp.tile([P, D], F16, name="kv")
                    for oh in range(2):
                        if with_bias:
                            nc.vector.tensor_add(
                                vv[:, 512 * oh:512 * oh + 512],
                                pss[oh][:],
                                bv[:, 512 * oh:512 * oh + 512],
                            )
                        else:
                            nc.scalar.copy(
                                vv[:, 512 * oh:512 * oh + 512], pss[oh][:])
                    for c0 in (0, 8):
                        nc.vector.tensor_mul(
                            prod2[:, :, c0:c0 + 8],
                            vv[:, 512 * (c0 // 8):512 * (c0 // 8) + 512]
                            .rearrange("p (c e) -> p e c", c=8),
                            attns[b][t][:, h, c0:c0 + 8][:, None, :]
                            .broadcast_to([P, 64, 8]),
                        )
                    part = smallp.tile([P, 64], F16, name="part")
                    with nc.allow_low_precision("single output rounding"):
                        nc.vector.tensor_reduce(
                            part[:], prod2[:],
                            axis=mybir.AxisListType.X, op=mybir.AluOpType.add,
                        )
                    parts.append(part)
                if pending is not None:
                    repack(*pending)
                pending = (bh, parts)
        repack(*pending)
        out_proj(3)

    return nc


_NC_CACHE = {}


def _get_nc(with_bias=True):
    if with_bias not in _NC_CACHE:
        _NC_CACHE[with_bias] = _build_nc(with_bias)
    return _NC_CACHE[with_bias]


def _host_prep(q, k, v, Wq, bq, Wk, bk, Wv, bv, Wo, bo):
    """Build the 8 per-core input maps (k fp8, v/q/weights bf16, biases f32)."""
    q = np.asarray(q, np.float32)
    k = np.asarray(k, np.float32)
    v = np.asarray(v, np.float32)
    # fold 1/sqrt(hd) into the q path (fp8 scales are undone on-device)
    Wqs = np.asarray(Wq, np.float32) * 0.125
    bqs = np.asarray(bq, np.float32) * 0.125

    def wprep(Wm, scale=1.0, dt=NPF16, clip=None):
        # [P, 8, D]: [f_local, f_chunk, o] of W.T
        WT = np.ascontiguousarray(np.asarray(Wm, np.float32).T) * scale  # [f, o]
        if clip is not None:
            WT = np.clip(WT, -clip, clip)
        return np.ascontiguousarray(
            WT.reshape(8, P, D).transpose(1, 0, 2)).astype(dt)

    wq_b = wprep(Wqs)
    wk_b = wprep(Wk, WK_SCALE, NPF8, 240.0)
    wv_b = wprep(Wv)
    wo_b = wprep(Wo)

    i_idx = np.arange(P)  # rows (j', i) packing for sel2
    r_i = i_idx % 16
    r_j = i_idx // 16
    g_col = np.arange(64) // 8
    j_col = np.arange(64) % 8
    # sel2[p=(j',i), q, (g,r)] = (j'==r) & (i == 2g+q)
    sel2 = np.stack([
        ((r_j[:, None] == j_col[None, :]) &
         (r_i[:, None] == 2 * g_col[None, :] + qq)).astype(NPF16)
        for qq in range(2)
    ], axis=1)  # [P, 2, 64]

    jj = np.arange(16)
    hh = np.arange(H)
    in_maps = []
    for c in range(NCORES):
        srows = hh[:, None] * 128 + c * 16 + jj[None, :]  # [h, j]

        def kvprep(x, dt, scale):
            xs = x[:, srows]  # [b, h, j, w, d]
            xs = xs.reshape(B, H, 16, 16, 2, D).transpose(0, 1, 4, 2, 3, 5)
            flat = xs.reshape(ROWS, D)  # rows (b,h,p,j,i)
            # kTb[f_local, bh_block, f_chunk, r'] where block rows r' in [0,512)
            xT = flat.T * scale  # [D, ROWS]
            if dt is NPF8:
                xT = np.clip(xT, -240.0, 240.0)
            xT = xT.astype(dt).reshape(8, P, NBH, 512)  # fc, f, blk, r
            return np.ascontiguousarray(xT.transpose(1, 2, 0, 3))  # f, blk, fc, r

        qc = q[:, c * 256:(c + 1) * 256, :].reshape(512, D)
        qT = qc.T.astype(NPF16).reshape(8, P, 512)
        qTb = np.ascontiguousarray(qT.transpose(1, 0, 2))  # [P, 8, 512]

        in_maps.append({
            "kTb": kvprep(k, NPF8, K_SCALE),
            "vTb": kvprep(v, NPF16, 1.0),
            "qTb": qTb,
            "wq": wq_b, "wk": wk_b, "wv": wv_b, "wo": wo_b,
            "bq": bqs,
            "bk": np.asarray(bk, np.float32),
            "bv": np.asarray(bv, np.float32), "bo": np.asarray(bo, np.float32),
            "sel2": sel2,
        })
    return in_maps


def kernel(q, k, v, Wq, bq, Wk, bk, Wv, bv, Wo, bo, num_heads, _trace=False):
    assert int(num_heads) == H
    with_bias = any(
        np.any(np.asarray(x, np.float32)) for x in (bq, bk, bv, bo))
    nc = _get_nc(with_bias)
    in_maps = _host_prep(q, k, v, Wq, bq, Wk, bk, Wv, bv, Wo, bo)
    res = run_bass_kernel_spmd(nc, in_maps, core_ids=list(range(NCORES)),
                               trace=_trace)
    full = np.zeros((B, S, D), np.float32)
    for c in range(NCORES):
        oc = res.results[c]["out"].reshape(B, H, 16, D)
        for h in range(H):
            full[:, h * 128 + c * 16: h * 128 + c * 16 + 16, :] = oc[:, h]
    if _trace:
        kernel._last_exec_ns = res.exec_time_ns
        kernel._last_results = res
    return full


# revision 33
# speedup vs baseline: 1.0021x; 1.0004x over previous
"""Trainium2 Bass kernel for nn_MultiHeadLocalAttention (B=2,S=2048,W=32,D=1024,H=16).

Sharding: query-chunk parallel over 8 cores. Core c handles query rows
s' in [c*256, (c+1)*256) for both batches. Because of the reference's raw
.view on the k/v projections, head h of query s' reads k rows
s = h*128 + s'//16, w = 2*(s' mod 16) + w'//16 — i.e. core c needs exactly
k/v rows {h*128 + c*16 + j : h in [0,16), j in [0,16)}, giving a perfect
8-way split of the dominant k/v projection GEMMs (34 GFLOP each per core).

Per-core device row order for k/v is (b, h, p, j, i) where the original
(s, w) maps as s = h*128 + c*16 + j, w = 2i + p. With that order:
  - scores row (p,j,i) needs q_proj row u = 128*jh + 16j' + i (j = 8jh+j'),
    which is the SAME partition index in the matching q tile — no gather.
  - softmax over h is a free-dim reduce over per-(b,h) score planes.
  - the p-pair sum of the attention-weighted v partials is a PSUM
    accumulation of parity-selector matmuls that simultaneously produce
    the transposed layout needed by the output projection.

The k projection runs in fp8-e4m3 with DoubleRow (2 fp8 MACs/PE cell,
measured ~1.9x over 16-bit): score errors are damped by the head-softmax
so e4m3 keeps rel err under the gate; the v path stays 16-bit (its
errors pass straight through to the output). k is scaled x16 and Wk x256
on the host to keep e4m3 quantization away from subnormals; the 1/4096
is undone by the PSUM->SBUF copy scale. All 16-bit tensors use fp16
(not bf16) for the 3 extra mantissa bits — same PE/DVE speed. The
1/sqrt(hd) scale is folded into Wq/bq on the host.
"""
import sys
import types

sys.path.insert(0, "/opt/trn_rl_repo")

import numpy as np
import ml_dtypes

import concourse.bass as bass
import concourse.mybir as mybir
import concourse.tile as tile
from concourse.bass_utils import run_bass_kernel_spmd

BF16 = mybir.dt.bfloat16
F16 = mybir.dt.float16
F32 = mybir.dt.float32
FP8 = mybir.dt.float8e4
NPBF = ml_dtypes.bfloat16
NPF16 = np.float16
NPF8 = ml_dtypes.float8_e4m3
DR = mybir.MatmulPerfMode.DoubleRow

B, S, W, D, H = 2, 2048, 32, 1024, 16
P = 128
NCORES = 8
ROWS = B * H * 2 * 16 * 16  # 16384 k/v rows per core, order (b,h,p,j,i)
NBH = B * H  # 32 (b,h) blocks of 512 rows

K_SCALE = 16.0    # k quantization pre-scale for e4m3
WK_SCALE = 256.0  # Wk quantization pre-scale for e4m3


class _TC(tile.TileContext):
    """Walrus in this env rejects instructions carrying >1 sem wait (its
    setupSyncWait has a single wait slot). Two fixes at context exit:
    1. split any multi-wait instruction: excess waits move to same-engine
       NOPs inserted immediately before it (per-engine stream order makes
       this equivalent);
    2. emit the exit-drain's waits as individual SP wait_ge instructions
       instead of on the Drain itself."""

    def _split_multi_waits(self):
        nc = self.nc
        eng = {
            mybir.EngineType.PE: nc.tensor,
            mybir.EngineType.DVE: nc.vector,
            mybir.EngineType.Activation: nc.scalar,
            mybir.EngineType.Pool: nc.gpsimd,
            mybir.EngineType.SP: nc.sync,
        }
        end_bb = nc.cur_bb.bb
        for f in nc.m.functions:
            for blk in f.blocks:
                snapshot = list(blk.instructions)
                inserts = {}
                created = []
                for idx, ins in enumerate(snapshot):
                    si = getattr(ins, "sync_info", None)
                    if si is None or not si.on_wait or len(si.on_wait) <= 1:
                        continue
                    waits = list(si.on_wait)
                    nops = []
                    for w in waits[:-1]:
                        assert w.wait_reg is None, "register wait can't be split"
                        b = eng[ins.engine].nop()  # appends to end_bb
                        b.ins.sync_info = mybir.SyncInfo(on_wait=[w], on_update=[])
                        nops.append(b.ins)
                        created.append(b.ins)
                    si.on_wait = waits[-1:]
                    inserts[idx] = nops
                if not inserts:
                    continue
                created_ids = {id(n) for n in created}
                # pull the freshly-appended nops back out of the end block
                end_bb.instructions = [
                    i for i in end_bb.instructions if id(i) not in created_ids
                ]
                out = []
                for idx, ins in enumerate(snapshot):
                    out.extend(inserts.get(idx, ()))
                    out.append(ins)
                blk.instructions = out

    def _drain_and_barrier(self, tick_clock, wait_clock):
        self._split_multi_waits()
        gc = tick_clock.global_clock
        for proc, sem in sorted(wait_clock.sems.allocated().items()):
            ticks = gc.peek_next(proc) - 1
            if ticks > 0:
                val = ticks * (16 if sem.name.startswith("DMA") else 1)
                self.nc.sync.wait_ge(sem, val)
        self.nc.sync.drain()
        self.nc.all_engine_barrier()
        popped = self.nc._tile_sem_poison_stack.pop()
        assert popped is self._sem_poison
        self.nc.clear_and_free_semaphores(list(self.sems.allocated().values()))
        self.nc.all_engine_barrier()


def _build_nc(with_bias=True):
    import contextlib

    nc = bass.Bass()
    kTb = nc.dram_tensor("kTb", [P, NBH, 8, 512], FP8, kind="ExternalInput")
    vTb = nc.dram_tensor("vTb", [P, NBH, 8, 512], F16, kind="ExternalInput")
    qTb = nc.dram_tensor("qTb", [P, 8, 512], F16, kind="ExternalInput")
    wqd = nc.dram_tensor("wq", [P, 8, D], F16, kind="ExternalInput")
    wkd = nc.dram_tensor("wk", [P, 8, D], FP8, kind="ExternalInput")
    wvd = nc.dram_tensor("wv", [P, 8, D], F16, kind="ExternalInput")
    wod = nc.dram_tensor("wo", [P, 8, D], F16, kind="ExternalInput")
    bqd = nc.dram_tensor("bq", [D], F32, kind="ExternalInput")
    bkd = nc.dram_tensor("bk", [D], F32, kind="ExternalInput")
    bvd = nc.dram_tensor("bv", [D], F32, kind="ExternalInput")
    bod = nc.dram_tensor("bo", [D], F32, kind="ExternalInput")
    sel2d = nc.dram_tensor("sel2", [P, 2, 64], F16, kind="ExternalInput")
    outd = nc.dram_tensor("out", [512, D], F16, kind="ExternalOutput")

    with _TC(nc) as tc, contextlib.ExitStack() as ex:
        wpool = ex.enter_context(tc.tile_pool(name="wts", bufs=1))
        persist = ex.enter_context(tc.tile_pool(name="persist", bufs=1))
        kblk = ex.enter_context(tc.tile_pool(name="kblk", bufs=4))
        vblk = ex.enter_context(tc.tile_pool(name="vblk", bufs=3))
        actp = ex.enter_context(tc.tile_pool(name="act", bufs=16))
        prodp = ex.enter_context(tc.tile_pool(name="prod", bufs=5))
        miscp = ex.enter_context(tc.tile_pool(name="misc", bufs=4))
        smallp = ex.enter_context(tc.tile_pool(name="small", bufs=12))
        gps = ex.enter_context(tc.tile_pool(name="gps", bufs=6 if not with_bias else 4,
                                            space="PSUM"))
        pps = ex.enter_context(tc.tile_pool(name="pps", bufs=2, space="PSUM"))

        # ---- constants -------------------------------------------------
        # chunked weight loads: one dma per f-chunk so transfers spread
        # across the 16 DMA queues and the first matmul isn't gated on a
        # single 2MB transfer draining through one queue
        def load_w(dram_t, name, dt=F16):
            t = wpool.tile([P, 8, D], dt, name=name)
            for fc in range(8):
                nc.sync.dma_start(t[:, fc], dram_t[:, fc])
            return t

        def load_b(dram_t, name):
            t = wpool.tile([P, D], F32, name=name)
            nc.sync.dma_start(t[:], dram_t[:].partition_broadcast(P))
            return t

        # DMA waits are cumulative in issue order, so emit only what the
        # q projection needs first; wk/sel before the K loop; wv/wo and
        # the v prefetch are interleaved into the K loop emission below.
        # partition-sliced loads: per-partition rows are contiguous in DRAM,
        # so a 16-partition slice is 16 fat descriptors instead of 128 thin
        # ones — the startup path is DMA-descriptor-bound, not byte-bound
        qts = persist.tile([P, 8, 512], F16, name="qts")
        for p0 in range(0, P, 16):
            nc.sync.dma_start(qts[p0:p0 + 16], qTb[p0:p0 + 16])
        wq = wpool.tile([P, 8, D], F16, name="wq")
        for p0 in range(0, P, 16):
            nc.sync.dma_start(wq[p0:p0 + 16], wqd[p0:p0 + 16])
        if with_bias:
            bq = load_b(bqd, "bq")

        # ---- q projection ---------------------------------------------
        qp = persist.tile([P, 4, D], F16, name="qp")  # u-chunks (b, jh)
        for t in range(4):
            for oh in range(2):
                ps = gps.tile([P, 512], F32, name="gps")
                for fc in range(8):
                    nc.tensor.matmul(
                        ps[:],
                        qts[:, fc, 128 * t:128 * t + 128],
                        wq[:, fc, 512 * oh:512 * oh + 512],
                        start=(fc == 0),
                        stop=(fc == 7),
                    )
                if with_bias:
                    nc.vector.tensor_add(
                        qp[:, t, 512 * oh:512 * oh + 512],
                        ps[:],
                        bq[:, 512 * oh:512 * oh + 512],
                    )
                else:
                    nc.scalar.copy(qp[:, t, 512 * oh:512 * oh + 512], ps[:])

        # ---- score planes / attn planes (persistent) -------------------
        planes = [[persist.tile([P, H, 16], F32, name=f"pl_{b}_{t}")
                   for t in range(4)] for b in range(B)]
        attns = [[persist.tile([P, H, 16], F16, name=f"at_{b}_{t}")
                  for t in range(4)] for b in range(B)]
        # transposed out_local, chunked by contraction block g and row-chunk:
        # olT[rc][cl, g, r] = out_local[row 128*rc + r, c = 128*g + cl]
        olTs = [persist.tile([P, 8, P], F16, name=f"olT{rc}")
                for rc in range(4)]

        kpre = {}
        for kb in range(3):
            tk = kblk.tile([P, 8, 512], FP8, name="kblk")
            nc.sync.dma_start(tk[:], kTb[:, kb])
            kpre[kb] = tk
        wk = load_w(wkd, "wk", FP8)
        sel2 = wpool.tile([P, 2, 64], F16, name="sel2")
        nc.sync.dma_start(sel2[:], sel2d[:])
        if with_bias:
            bk = load_b(bkd, "bk")
        vpre = {}

        # ---- K phase: fp8 DoubleRow projections + scores ---------------
        for b in range(B):
            for h in range(H):
                bh = b * H + h
                if bh in kpre:
                    ts_ = kpre.pop(bh)
                else:
                    ts_ = kblk.tile([P, 8, 512], FP8, name="kblk")
                    nc.sync.dma_start(ts_[:], kTb[:, bh])
                if bh == 2:
                    # prefetch the first v blocks + V-phase weights now:
                    # late enough not to gate the K start, early enough
                    # to be resident long before the V phase begins
                    for vb in range(2):
                        tv = vblk.tile([P, 8, 512], F16, name="vblk")
                        nc.sync.dma_start(tv[:], vTb[:, vb])
                        vpre[vb] = tv
                    wv = load_w(wvd, "wv")
                    wo = load_w(wod, "wo")
                    if with_bias:
                        bv = load_b(bvd, "bv")
                        bo = load_b(bod, "bo")
                for t in range(4):
                    pss = [gps.tile([P, 512], F32, name="gps")
                           for _ in range(2)]
                    for fcp in range(4):
                        for oh in range(2):
                            nc.tensor.matmul(
                                pss[oh],
                                ts_[:, 2 * fcp:2 * fcp + 2,
                                    128 * t:128 * t + 128],
                                wk[:, 2 * fcp:2 * fcp + 2,
                                   512 * oh:512 * oh + 512],
                                start=(fcp == 0),
                                stop=(fcp == 3),
                                perf_mode=DR,
                            )
                    kv = actp.tile([P, D], F16, name="kv")
                    for oh in range(2):
                        if with_bias:
                            nc.vector.scalar_tensor_tensor(
                                kv[:, 512 * oh:512 * oh + 512],
                                pss[oh][:],
                                1.0 / (K_SCALE * WK_SCALE),
                                bk[:, 512 * oh:512 * oh + 512],
                                op0=mybir.AluOpType.mult,
                                op1=mybir.AluOpType.add,
                            )
                        else:
                            nc.scalar.mul(
                                kv[:, 512 * oh:512 * oh + 512], pss[oh][:],
                                1.0 / (K_SCALE * WK_SCALE))
                    jh = t % 2
                    qsm = (qp[:, 2 * b + jh, 64 * h:64 * h + 64][:, None, :])
                    prod = prodp.tile([P, 16, 64], F16, name="prod")
                    nc.vector.tensor_mul(
                        prod[:],
                        kv[:].rearrange("p (c e) -> p c e", c=16),
                        qsm.broadcast_to([P, 16, 64]),
                    )
                    nc.vector.tensor_reduce(
                        planes[b][t][:, h, :],
                        prod[:],
                        axis=mybir.AxisListType.X,
                        op=mybir.AluOpType.add,
                    )

        # ---- softmax over heads ----------------------------------------
        for b in range(B):
            for t in range(4):
                pl = planes[b][t]
                mx = smallp.tile([P, 16], F32, name="mx")
                nc.vector.tensor_reduce(
                    mx[:], pl[:].rearrange("p h c -> p c h"),
                    axis=mybir.AxisListType.X, op=mybir.AluOpType.max,
                )
                sub = miscp.tile([P, H, 16], F32, name="sm")
                nc.vector.tensor_sub(
                    sub[:], pl[:],
                    mx[:][:, None, :].broadcast_to([P, H, 16]),
                )
                epl = miscp.tile([P, H, 16], F32, name="ep")
                nc.scalar.activation(
                    epl[:], sub[:], mybir.ActivationFunctionType.Exp,
                )
                z = smallp.tile([P, 16], F32, name="z")
                nc.vector.tensor_reduce(
                    z[:], epl[:].rearrange("p h c -> p c h"),
                    axis=mybir.AxisListType.X, op=mybir.AluOpType.add,
                )
                rz = smallp.tile([P, 16], F32, name="rz")
                nc.vector.reciprocal(rz[:], z[:])
                nc.vector.tensor_mul(
                    attns[b][t][:], epl[:],
                    rz[:][:, None, :].broadcast_to([P, H, 16]),
                )

        # ---- V phase: fp16 projections + weighted sum + repack ---------
        def out_proj(rc):
            for oh in range(2):
                ps = gps.tile([P, 512], F32, name="gps")
                for g in range(8):
                    nc.tensor.matmul(
                        ps[:],
                        olTs[rc][:, g, :],
                        wo[:, g, 512 * oh:512 * oh + 512],
                        start=(g == 0),
                        stop=(g == 7),
                    )
                for half in range(2):
                    osb = miscp.tile([P, 256], F16, name="osb")
                    sl = slice(256 * half, 256 * half + 256)
                    if with_bias:
                        nc.vector.tensor_add(
                            osb[:], ps[:, sl],
                            bo[:, 512 * oh + 256 * half:
                                512 * oh + 256 * half + 256])
                    else:
                        nc.scalar.copy(osb[:], ps[:, sl])
                    nc.sync.dma_start(
                        outd[128 * rc:128 * rc + 128,
                             512 * oh + 256 * half:
                             512 * oh + 256 * half + 256],
                        osb[:],
                    )

        def repack(bh, parts):
            # emitted one block late: the PE streams the next block's GEMM
            # while the DVE finishes this block's parts
            for jh in range(2):
                aps = pps.tile([P, 64], F32, name="pps")
                for ti, t in enumerate((jh, 2 + jh)):
                    for q3 in range(2):
                        nc.tensor.matmul(
                            aps[:][64 * q3:64 * q3 + 64, :],
                            parts[t][:],
                            sel2[:, q3, :],
                            start=(ti == 0),
                            stop=(ti == 1),
                            tile_position=(0, 64 * q3),
                            skip_group_check=True,
                        )
                rb = bh * 16 + jh * 8
                nc.scalar.copy(
                    olTs[rb // P][:, :, rb % P:rb % P + 8],
                    aps[:].rearrange("p (g r) -> p g r", g=8),
                )

        pending = None
        for b in range(B):
            for h in range(H):
                bh = b * H + h
                if bh in vpre:
                    ts_ = vpre.pop(bh)
                else:
                    ts_ = vblk.tile([P, 8, 512], F16, name="vblk")
                    nc.sync.dma_start(ts_[:], vTb[:, bh])
                # delayed output projection: emit one block late so the PE
                # streams this block's GEMM while Act finishes olT copies
                if bh % 8 == 1 and bh > 8:
                    out_proj(bh // 8 - 1)
                parts = []
                for t in range(4):
                    pss = [gps.tile([P, 512], F32, name="gps")
                           for _ in range(2)]
                    for fc in range(8):
                        for oh in range(2):
                            nc.tensor.matmul(
                                pss[oh],
                                ts_[:, fc, 128 * t:128 * t + 128],
                                wv[:, fc, 512 * oh:512 * oh + 512],
                                start=(fc == 0),
                                stop=(fc == 7),
                            )
                    prod2 = prodp.tile([P, 64, 16], F16, name="pr2")
                    vv = actp.tile([P, D], F16, name="kv")
                    for oh in range(2):
                        if with_bias:
                            nc.vector.tensor_add(
                                vv[:, 512 * oh:512 * oh + 512],
                                pss[oh][:],
                                bv[:, 512 * oh:512 * oh + 512],
                            )
                        else:
                            nc.scalar.copy(
                                vv[:, 512 * oh:512 * oh + 512], pss[oh][:])
                    for c0 in (0, 8):
                        nc.vector.tensor_mul(
                            prod2[:, :, c0:c0 + 8],
                            vv[:, 512 * (c0 // 8):512 * (c0 // 8) + 512]
                            .rearrange("p (c e) -> p e c", c=8),
                            attns[b][t][:, h, c0:c0 + 8][:, None, :]
                            .broadcast_to([P, 64, 8]),
                        )
                    part = smallp.tile([P, 64], F16, name="part")
                    with nc.allow_low_precision("single output rounding"):
                        nc.vector.tensor_reduce(
                            part[:], prod2[:],
                            axis=mybir.AxisListType.X, op=mybir.AluOpType.add,
                        )
                    parts.append(part)
                if pending is not None:
                    repack(*pending)
                pending = (bh, parts)
        repack(*pending)
        out_proj(3)

    return nc


_NC_CACHE = {}


def _get_nc(with_bias=True):
    if with_bias not in _NC_CACHE:
        _NC_CACHE[with_bias] = _build_nc(with_bias)
    return _NC_CACHE[with_bias]


def _host_prep(q, k, v, Wq, bq, Wk, bk, Wv, bv, Wo, bo):
    """Build the 8 per-core input maps (k fp8, v/q/weights bf16, biases f32)."""
    q = np.asarray(q, np.float32)
    k = np.asarray(k, np.float32)
    v = np.asarray(v, np.float32)
    # fold 1/sqrt(hd) into the q path (fp8 scales are undone on-device)
    Wqs = np.asarray(Wq, np.float32) * 0.125
    bqs = np.asarray(bq, np.float32) * 0.125

    def wprep(Wm, scale=1.0, dt=NPF16, clip=None):
        # [P, 8, D]: [f_local, f_chunk, o] of W.T
        WT = np.ascontiguousarray(np.asarray(Wm, np.float32).T) * scale  # [f, o]
        if clip is not None:
            WT = np.clip(WT, -clip, clip)
        return np.ascontiguousarray(
            WT.reshape(8, P, D).transpose(1, 0, 2)).astype(dt)

    wq_b = wprep(Wqs)
    wk_b = wprep(Wk, WK_SCALE, NPF8, 240.0)
    wv_b = wprep(Wv)
    wo_b = wprep(Wo)

    i_idx = np.arange(P)  # rows (j', i) packing for sel2
    r_i = i_idx % 16
    r_j = i_idx // 16
    g_col = np.arange(64) // 8
    j_col = np.arange(64) % 8
    # sel2[p=(j',i), q, (g,r)] = (j'==r) & (i == 2g+q)
    sel2 = np.stack([
        ((r_j[:, None] == j_col[None, :]) &
         (r_i[:, None] == 2 * g_col[None, :] + qq)).astype(NPF16)
        for qq in range(2)
    ], axis=1)  # [P, 2, 64]

    jj = np.arange(16)
    hh = np.arange(H)
    in_maps = []
    for c in range(NCORES):
        srows = hh[:, None] * 128 + c * 16 + jj[None, :]  # [h, j]

        def kvprep(x, dt, scale):
            xs = x[:, srows]  # [b, h, j, w, d]
            xs = xs.reshape(B, H, 16, 16, 2, D).transpose(0, 1, 4, 2, 3, 5)
            flat = xs.reshape(ROWS, D)  # rows (b,h,p,j,i)
            # kTb[f_local, bh_block, f_chunk, r'] where block rows r' in [0,512)
            xT = flat.T * scale  # [D, ROWS]
            if dt is NPF8:
                xT = np.clip(xT, -240.0, 240.0)
            xT = xT.astype(dt).reshape(8, P, NBH, 512)  # fc, f, blk, r
            return np.ascontiguousarray(xT.transpose(1, 2, 0, 3))  # f, blk, fc, r

        qc = q[:, c * 256:(c + 1) * 256, :].reshape(512, D)
        qT = qc.T.astype(NPF16).reshape(8, P, 512)
        qTb = np.ascontiguousarray(qT.transpose(1, 0, 2))  # [P, 8, 512]

        in_maps.append({
            "kTb": kvprep(k, NPF8, K_SCALE),
            "vTb": kvprep(v, NPF16, 1.0),
            "qTb": qTb,
            "wq": wq_b, "wk": wk_b, "wv": wv_b, "wo": wo_b,
            "bq": bqs,
            "bk": np.asarray(bk, np.float32),
            "bv": np.asarray(bv, np.float32), "bo": np.asarray(bo, np.float32),
            "sel2": sel2,
        })
    return in_maps


def kernel(q, k, v, Wq, bq, Wk, bk, Wv, bv, Wo, bo, num_heads, _trace=False):
    assert int(num_heads) == H
    with_bias = any(
        np.any(np.asarray(x, np.float32)) for x in (bq, bk, bv, bo))
    nc = _get_nc(with_bias)
    in_maps = _host_prep(q, k, v, Wq, bq, Wk, bk, Wv, bv, Wo, bo)
    res = run_bass_kernel_spmd(nc, in_maps, core_ids=list(range(NCORES)),
                               trace=_trace)
    full = np.zeros((B, S, D), np.float32)
    for c in range(NCORES):
        oc = res.results[c]["out"].reshape(B, H, 16, D)
        for h in range(H):
            full[:, h * 128 + c * 16: h * 128 + c * 16 + 16, :] = oc[:, h]
    if _trace:
        kernel._last_exec_ns = res.exec_time_ns
        kernel._last_results = res
    return full
